# revision 2
# baseline (speedup 1.0000x reference)
"""GAT (3-layer graph attention + final linear) Trainium2 Bass kernel, v2.

Problem: B=4 graphs, N=2048 atoms, D=128, H=256.
  per layer: h = relu(x @ W.T + b); e_ij = leaky_relu(f1_i + f2_j, 0.01)
  masked by adj; att = softmax_j(e); x = x + att @ h.
  final: relu(x @ Wt.T + bt).

Algorithmic core (validated vs reference at ~4e-3 rel err in numpy):
  exp(leaky_relu(z)) ~= e^z + e^{0.01 z}  (exact at both tails, <=2x off
  only near z=0 where softmax mass is negligible). With z = f1_i + f2_j
  the masked softmax becomes a sum of two rank-1 terms:
    att@h = (num1 + w*num2) / (den1 + w*den2),   w_i = exp(-0.99 f1_i)
    num1 = adj @ (v (.) h), den1 = adj @ v,   v  = exp(f2)
    num2 = adj @ (v'(.) h), den2 = adj @ v',  v' = exp(0.01 f2)
  No NxN exp/mask work remains; the only NxN operand is the adjacency,
  constant across layers, uploaded once (bf16 + fp8 copies, exact 0/1).

Sharding: core c -> (graph c//2, row-half c%2). The SPMD program is
parity-agnostic: the host permutes each core's adjacency columns so its
OWN atoms are j-tiles 0..7 and the partner's are 8..15. Per layer each
core computes h for its own rows (f32), pair-AllReduce(add)s it, and
recovers the partner half by subtraction -- identical code on all cores.
The exchange is hidden under own-half attention work.

Engine mapping:
  PE:   h matmul, f1/f2, num1/den (bf16), num2 (fp8 DoubleRow, two
        j-tiles per pass), small row->col transposes for f and den.
  DMA:  XBAR transposes (hsT->hnat, numT->nat, x->xT, final out).
  DVE:  relu+bias, v-scalings of h (stride-0 broadcast APs), batched
        normalize/residual update.
  ACT:  exps only (single LUT set).
"""

import numpy as np
import ml_dtypes

import concourse.bass as bass
import concourse.mybir as mybir
import concourse.tile as tile
from concourse import masks
from concourse.bass_utils import run_bass_kernel_spmd

P = 128
F32 = mybir.dt.float32
BF16 = mybir.dt.bfloat16
FP8 = mybir.dt.float8e4
F16 = mybir.dt.float16
AF = mybir.ActivationFunctionType
OP = mybir.AluOpType
DR = mybir.MatmulPerfMode.DoubleRow
BFNP = ml_dtypes.bfloat16
FP8NP = ml_dtypes.float8_e4m3


def _legalize_waits(nc, dma_limit=1, engine_limit=1):
    """Walrus encodes few sem waits per instr (1 on DMA, 0 on XPOSE DMA,
    ~2 on engine instrs). Move excess waits onto standalone EventSemaphore
    instructions on the same engine."""
    counter = [0]

    def split(ins):
        si = ins.sync_info
        if si is None:
            return None
        tname = type(ins).__name__
        if tname == "InstDmaTransposeAnt":
            limit = 0
        elif tname.startswith("InstDMA") or tname.startswith("InstDma"):
            limit = dma_limit
        else:
            limit = engine_limit
        waits = list(si.on_wait)
        if len(waits) <= limit:
            return None
        keep = waits[-limit:] if limit > 0 else []
        extra = waits[:-limit] if limit > 0 else waits
        evs = []
        for w in extra:
            counter[0] += 1
            evs.append(mybir.InstEventSemaphore(
                name=f"evsplit{counter[0]}", engine=ins.engine,
                sync_info=mybir.SyncInfo(on_wait=[w], on_update=[])))
        ins.sync_info = mybir.SyncInfo(on_wait=keep,
                                       on_update=list(si.on_update))
        return evs

    for f in nc.m.functions:
        for blk in f.blocks:
            new_list = []
            changed = False
            for ins in blk.instructions:
                evs = split(ins)
                if evs:
                    new_list.extend(evs)
                    changed = True
                new_list.append(ins)
            if changed:
                blk.instructions = new_list


def build_gat_nc(N, NS, D, H, num_cores, pair_groups, nlayers=3,
                 legalize=True):
    assert D == P
    nj = N // P          # 16 j tiles (own-first per-core labeling)
    nit = NS // P        # 8 i tiles / own j tiles
    npair = nj // 2      # 8 DoubleRow j-tile pairs
    nH = H // P

    nc = bass.Bass("TRN2", target_bir_lowering=False, debug=False,
                   num_devices=num_cores)

    # ---- I/O ----
    adjTb_in = nc.dram_tensor("adjTb", [N, NS], BF16, kind="ExternalInput")
    adjT8_in = nc.dram_tensor("adjT8", [N, NS], FP8, kind="ExternalInput")
    xTs_in = nc.dram_tensor("xTs", [P, N], BF16, kind="ExternalInput")
    xs_in = nc.dram_tensor("xs", [NS, D], F32, kind="ExternalInput")
    WT_in = [nc.dram_tensor(f"WT{l}", [D, D], BF16, kind="ExternalInput")
             for l in range(nlayers)]
    bv_in = [nc.dram_tensor(f"bv{l}", [D, 1], F32, kind="ExternalInput")
             for l in range(nlayers)]
    av_in = [nc.dram_tensor(f"av{l}", [D, 2], BF16, kind="ExternalInput")
             for l in range(nlayers)]
    WtT_in = nc.dram_tensor("WtT", [D, H], BF16, kind="ExternalInput")
    btp_in = nc.dram_tensor("btp", [P, nH], F32, kind="ExternalInput")
    out_ext = nc.dram_tensor("out_s", [NS, H], BF16, kind="ExternalOutput")

    # DRAM bounce buffers for the pair AllGather of x halves (bf16).
    # Parity-free partner recovery: partner = slotA + slotB - own (exact
    # in f32 arithmetic on bf16 values). Layer 0 needs no exchange: both
    # halves of the initial x are uploaded.
    ag_in = [nc.dram_tensor(f"ag_in{l}", [P, NS], BF16)
             for l in range(1, nlayers)]
    ag_out = [nc.dram_tensor(f"ag_out{l}", [2 * P, NS], BF16)
              for l in range(1, nlayers)]

    with tile.TileContext(nc) as tc:
        import contextlib
        ctx = contextlib.ExitStack()
        with ctx:
            persist = ctx.enter_context(tc.tile_pool(name="persist", bufs=1))
            hsp = ctx.enter_context(tc.tile_pool(name="hsp", bufs=2))
            natp = ctx.enter_context(tc.tile_pool(name="natp", bufs=2))
            xsp = ctx.enter_context(tc.tile_pool(name="xsp", bufs=2))
            xtp = ctx.enter_context(tc.tile_pool(name="xtp", bufs=2))
            smallp = ctx.enter_context(tc.tile_pool(name="smallp", bufs=2))
            nsbp = ctx.enter_context(tc.tile_pool(name="nsbp", bufs=2))
            osbp = ctx.enter_context(tc.tile_pool(name="osbp", bufs=2))
            poolA = ctx.enter_context(
                tc.tile_pool(name="poolA", bufs=2, space="PSUM"))
            pn1 = ctx.enter_context(
                tc.tile_pool(name="pn1", bufs=1, space="PSUM"))
            pn2 = ctx.enter_context(
                tc.tile_pool(name="pn2", bufs=1, space="PSUM"))
            pden = ctx.enter_context(
                tc.tile_pool(name="pden", bufs=1, space="PSUM"))

            ident = persist.tile([P, P], F32)
            masks.make_identity(nc, ident[:])

            # ---- persistent data (small tensors first: layer-0's h
            # matmul must not wait behind the 6MB adjacency upload) ----
            WT = [persist.tile([D, D], BF16, name=f"WT{l}", tag=f"WT{l}")
                  for l in range(nlayers)]
            bv = [persist.tile([D, 1], F32, name=f"bv{l}", tag=f"bv{l}")
                  for l in range(nlayers)]
            av = [persist.tile([D, 2], BF16, name=f"av{l}", tag=f"av{l}")
                  for l in range(nlayers)]
            xTs = [xtp.tile([P, NS], BF16, name=f"xTs0_{hh}",
                            tag=f"xTs{hh}") for hh in range(2)]
            nc.sync.dma_start(xTs[0][:], xTs_in.ap()[:, 0:NS])
            nc.sync.dma_start(xTs[1][:], xTs_in.ap()[:, NS:N])
            for l in range(nlayers):
                nc.sync.dma_start(WT[l][:], WT_in[l].ap())
                nc.sync.dma_start(bv[l][:], bv_in[l].ap())
                nc.sync.dma_start(av[l][:], av_in[l].ap())

            # per-quarter adjacency tiles: dependency tracking is
            # tile-granular, so aggregation on j-tile q must not wait for
            # later quarters' uploads
            adjTb = [persist.tile([P, 4, NS], BF16, name=f"adjTb{q}",
                                  tag=f"adjTb{q}") for q in range(nj // 4)]
            adjT8 = [persist.tile([P, 4, NS], FP8, name=f"adjT8{q}",
                                  tag=f"adjT8{q}") for q in range(nj // 4)]
            adjb_src = adjTb_in.ap().rearrange("(a p) i -> p a i", p=P)
            adj8_src = adjT8_in.ap().rearrange("(a p) i -> p a i", p=P)
            for q in range(nj // 4):
                sl = slice(4 * q, 4 * q + 4)
                nc.sync.dma_start(adjTb[q][:], adjb_src[:, sl, :])
                nc.sync.dma_start(adjT8[q][:], adj8_src[:, sl, :])

            # needed only mid-layer-0 / at the end: after the adj bulk
            xs = xsp.tile([P, nit, P], F32, name="xs0", tag="xs")
            nc.sync.dma_start(
                xs[:], xs_in.ap().rearrange("(a p) d -> p a d", p=P))
            WtT = persist.tile([D, H], BF16)
            nc.sync.dma_start(WtT[:], WtT_in.ap())
            btp = persist.tile([P, nH], F32)
            nc.sync.dma_start(btp[:], btp_in.ap())

            slA = slB = None
            for l in range(nlayers):
                hsT = [hsp.tile([P, NS], BF16, name=f"hsT{l}_{hh}",
                                tag=f"hsT{hh}") for hh in range(2)]
                hnat = [natp.tile([P, nit, P], BF16, name=f"hnat{l}_{hh}",
                                  tag=f"hnat{hh}") for hh in range(2)]
                g1 = [natp.tile([P, nit, P], BF16, name=f"g1_{l}_{hh}",
                                tag=f"g1_{hh}") for hh in range(2)]
                g2 = [natp.tile([P, nit, P], FP8, name=f"g2_{l}_{hh}",
                                tag=f"g2_{hh}") for hh in range(2)]
                frow = [smallp.tile([2, NS], F32, name=f"frow{l}_{hh}",
                                    tag=f"frow{hh}") for hh in range(2)]
                fcol = [smallp.tile([P, nit, 2], F32, name=f"fcol{l}_{hh}",
                                    tag=f"fcol{hh}") for hh in range(2)]
                vv = [smallp.tile([P, nit, 2], BF16, name=f"vv{l}_{hh}",
                                  tag=f"vv{hh}") for hh in range(2)]

                psn1 = pn1.tile([P, NS], F32, name=f"psn1_{l}", tag="n1")
                psn2 = pn2.tile([P, NS], F32, name=f"psn2_{l}", tag="n2")
                psden = pden.tile([2, NS], F32, name=f"psden{l}", tag="den")

                def h_half(hh):
                    """hsT[hh] = relu(WT^T @ xTs[hh] + b), bf16."""
                    for c in range(2):
                        src_ap = xTs[hh][:, c * 512:(c + 1) * 512]
                        ph = poolA.tile([P, 512], F32,
                                        name=f"ph{l}_{hh}_{c}", tag="A")
                        nc.tensor.matmul(ph[:], WT[l][:], src_ap,
                                         start=True, stop=True)
                        nc.vector.tensor_scalar(
                            hsT[hh][:, c * 512:(c + 1) * 512], ph[:],
                            bv[l][:], 0.0, OP.add, OP.max)

                def half_prep(hh):
                    """f1f2 + row->col transposes (PE) + exps + hnat + g
                    scalings for half hh (0 = own rows, 1 = partner)."""
                    for c in range(2):
                        pf = poolA.tile([2, 512], F32,
                                        name=f"pf{l}_{hh}_{c}", tag="A")
                        nc.tensor.matmul(
                            pf[:], av[l][:],
                            hsT[hh][:, c * 512:(c + 1) * 512],
                            start=True, stop=True)
                        nc.scalar.activation(
                            frow[hh][:, c * 512:(c + 1) * 512], pf[:],
                            AF.Copy)
                    pt = poolA.tile([P, 16], F32, name=f"pt{l}_{hh}",
                                    tag="A")
                    for q in range(nit):
                        nc.tensor.transpose(
                            pt[:, 2 * q:2 * q + 2],
                            frow[hh][:, q * P:(q + 1) * P], ident[0:2, 0:2])
                    nc.scalar.activation(
                        fcol[hh][:].rearrange("p a b -> p (a b)"), pt[:],
                        AF.Copy)
                    nc.scalar.activation(vv[hh][:, :, 0],
                                         fcol[hh][:, :, 1], AF.Exp)
                    nc.scalar.activation(vv[hh][:, :, 1],
                                         fcol[hh][:, :, 1], AF.Exp,
                                         scale=0.01)
                    nc.sync.dma_start(hnat[hh][:], hsT[hh][:],
                                      transpose=True)
                    vb = vv[hh][:, :, 0:1].broadcast_to([P, nit, P])
                    nc.vector.tensor_tensor(g1[hh][:], hnat[hh][:], vb,
                                            OP.mult)
                    vpb = vv[hh][:, :, 1:2].broadcast_to([P, nit, P])
                    nc.vector.tensor_tensor(g2[hh][:], hnat[hh][:], vpb,
                                            OP.mult)

                def half_agg(hh, first, last):
                    """num1/den bf16 + num2 fp8-DR streams for half hh."""
                    for q in range(nit):
                        aq, aj = (2 * hh + q // 4), q % 4
                        for c in range(2):
                            sl = slice(c * 512, (c + 1) * 512)
                            nc.tensor.matmul(
                                psn1[:, sl], g1[hh][:, q, :],
                                adjTb[aq][:, aj, sl],
                                start=(first and q == 0),
                                stop=(last and q == nit - 1))
                            nc.tensor.matmul(
                                psden[:, sl], vv[hh][:, q, :],
                                adjTb[aq][:, aj, sl],
                                start=(first and q == 0),
                                stop=(last and q == nit - 1))
                    for k in range(npair // 2):
                        aq, ak = (2 * hh + k // 2), k % 2
                        for c in range(2):
                            sl = slice(c * 512, (c + 1) * 512)
                            nc.tensor.matmul(
                                psn2[:, sl],
                                g2[hh][:, 2 * k:2 * k + 2, :],
                                adjT8[aq][:, 2 * ak:2 * ak + 2, sl],
                                start=(first and k == 0),
                                stop=(last and k == npair // 2 - 1),
                                perf_mode=DR)

                # own half first (overlaps the partner-x exchange that was
                # launched at the end of the previous layer), then partner
                h_half(0)
                half_prep(0)
                half_agg(0, first=True, last=False)
                if l > 0:
                    # partner x: xTs[1] = slA + slB - own  (exact on bf16)
                    nc.vector.tensor_tensor(slA[:], slA[:], slB[:], OP.add)
                    nc.vector.tensor_tensor(xTs[1][:], slA[:], xTs[0][:],
                                            OP.subtract)
                h_half(1)
                half_prep(1)
                half_agg(1, first=False, last=True)

                # num psum -> bf16 sbuf first (longest downstream chain)
                nsb1 = nsbp.tile([P, NS], BF16, name=f"nsb1_{l}",
                                 tag="nsb1")
                nc.scalar.activation(nsb1[:], psn1[:], AF.Copy)
                nsb2 = nsbp.tile([P, NS], BF16, name=f"nsb2_{l}",
                                 tag="nsb2")
                nc.vector.tensor_copy(nsb2[:], psn2[:])
                n1nat = natp.tile([P, nit, P], BF16, name=f"n1nat{l}",
                                  tag="n1nat")
                nc.sync.dma_start(n1nat[:], nsb1[:], transpose=True)
                n2nat = natp.tile([P, nit, P], BF16, name=f"n2nat{l}",
                                  tag="n2nat")
                nc.sync.dma_start(n2nat[:], nsb2[:], transpose=True)

                # -- den: psum [2, NS] -> dencol [P, nit, 2] --
                denrow = smallp.tile([2, NS], F32, name=f"denrow{l}",
                                     tag="denrow")
                nc.scalar.activation(denrow[:], psden[:], AF.Copy)
                pd = poolA.tile([P, 16], F32, name=f"pd{l}", tag="A")
                for q in range(nit):
                    nc.tensor.transpose(pd[:, 2 * q:2 * q + 2],
                                        denrow[:, q * P:(q + 1) * P],
                                        ident[0:2, 0:2])
                dencol = smallp.tile([P, nit, 2], F32, name=f"dencol{l}",
                                     tag="dencol")
                nc.scalar.activation(
                    dencol[:].rearrange("p a b -> p (a b)"), pd[:], AF.Copy)
                wcol = smallp.tile([P, nit], F32, name=f"wcol{l}",
                                   tag="wcol")
                nc.scalar.activation(wcol[:], fcol[0][:, :, 0],
                                     AF.Exp, scale=-0.99)

                # -- r = 1/(den1 + w den2), rw = r*w --
                dtot = smallp.tile([P, nit], F32, name=f"dtot{l}",
                                   tag="dtot")
                nc.vector.tensor_tensor(dtot[:], dencol[:, :, 1], wcol[:],
                                        OP.mult)
                nc.vector.tensor_tensor(dtot[:], dtot[:], dencol[:, :, 0],
                                        OP.add)
                rcol = smallp.tile([P, nit], F32, name=f"rcol{l}",
                                   tag="rcol")
                nc.vector.reciprocal(rcol[:], dtot[:])
                rwcol = smallp.tile([P, nit], F32, name=f"rwcol{l}",
                                    tag="rwcol")
                nc.vector.tensor_tensor(rwcol[:], rcol[:], wcol[:], OP.mult)

                # -- x update (batched, stride-0 free-dim broadcasts) --
                t1 = xsp.tile([P, nit, P], F32, name=f"t1_{l}", tag="t1")
                nc.vector.tensor_tensor(
                    t1[:], n1nat[:],
                    rcol[:].unsqueeze(2).broadcast_to([P, nit, P]), OP.mult)
                t2 = xsp.tile([P, nit, P], F32, name=f"t2_{l}", tag="t2")
                nc.vector.tensor_tensor(
                    t2[:], n2nat[:],
                    rwcol[:].unsqueeze(2).broadcast_to([P, nit, P]),
                    OP.mult)
                nc.vector.tensor_tensor(t1[:], t1[:], t2[:], OP.add)
                xs_new = xsp.tile([P, nit, P], F32, name=f"xs{l + 1}",
                                  tag="xs")
                nc.vector.tensor_tensor(xs_new[:], t1[:], xs[:], OP.add)
                xs = xs_new

                # -- xTs for next layer / final --
                xb = xtp.tile([P, nit, P], BF16, name=f"xb{l}", tag="xb")
                nc.scalar.activation(
                    xb[:].rearrange("p a b -> p (a b)"),
                    xs[:].rearrange("p a b -> p (a b)"), AF.Copy)
                xTs_new = xtp.tile([P, NS], BF16, name=f"xTs{l + 1}",
                                   tag="xTs0")
                nc.sync.dma_start(
                    xTs_new[:].rearrange("p (a b) -> p a b", b=P),
                    xb[:].rearrange("p a b -> p (a b)"), transpose=True)

                if l < nlayers - 1:
                    slA = hsp.tile([P, NS], BF16, name=f"slA{l + 1}",
                                   tag="slA")
                    slB = hsp.tile([P, NS], BF16, name=f"slB{l + 1}",
                                   tag="slB")
                    nc.gpsimd.dma_start(ag_in[l].ap(), xTs_new[:])
                    nc.gpsimd.collective_compute(
                        "AllGather", OP.bypass, replica_groups=pair_groups,
                        ins=[ag_in[l].ap()], outs=[ag_out[l].ap()])
                    nc.gpsimd.dma_start(slA[:], ag_out[l].ap()[0:P, :])
                    nc.gpsimd.dma_start(slB[:], ag_out[l].ap()[P:2 * P, :])
                    xTs = [xTs_new,
                           xtp.tile([P, NS], BF16, name=f"xTs{l + 1}_1",
                                    tag="xTs1")]
                else:
                    xTs = [xTs_new, None]

            # ---- final linear, transposed: outT = relu(WtT^T @ xTs + bt)
            onat = natp.tile([P, nit, nH, P], BF16, name="onat", tag="onat")
            for t in range(nH):
                osbT = osbp.tile([P, NS], BF16, name=f"osbT{t}",
                                 tag=f"osbT{t}")
                for c in range(2):
                    po = poolA.tile([P, 512], F32, name=f"po{t}_{c}",
                                    tag="A")
                    nc.tensor.matmul(po[:], WtT[:, t * P:(t + 1) * P],
                                     xTs[0][:, c * 512:(c + 1) * 512],
                                     start=True, stop=True)
                    nc.vector.tensor_scalar(osbT[:, c * 512:(c + 1) * 512],
                                            po[:], btp[:, t:t + 1], 0.0,
                                            OP.add, OP.max)
                nc.sync.dma_start(onat[:, :, t, :], osbT[:],
                                  transpose=True)
            nc.sync.dma_start(
                out_ext.ap().rearrange("(a p) (t q) -> p a t q", p=P, q=P),
                onat[:])

    if legalize:
        _legalize_waits(nc)
    return nc


def make_in_maps(x, adj, Ws, bs, avs, Wt, bt, num_cores, NS):
    """Per-core input dicts. Core c -> (graph c//2, row-half c%2).
    adjT columns (j) are permuted own-rows-first per core."""
    B, N, D = x.shape
    H = Wt.shape[0]
    x = np.ascontiguousarray(np.asarray(x), np.float32)
    adj = np.asarray(adj)
    shared = {
        "WtT": np.ascontiguousarray(
            np.asarray(Wt, np.float32).T).astype(BFNP),
        "btp": np.ascontiguousarray(
            np.asarray(bt, np.float32).reshape(H // P, P).T),
    }
    for l, (W, b, a) in enumerate(zip(Ws, bs, avs)):
        shared[f"WT{l}"] = np.ascontiguousarray(
            np.asarray(W, np.float32).T).astype(BFNP)
        shared[f"bv{l}"] = np.ascontiguousarray(
            np.asarray(b, np.float32).reshape(D, 1))
        a = np.asarray(a, np.float32)
        shared[f"av{l}"] = np.ascontiguousarray(
            np.stack([a[:D, 0], a[D:, 0]], axis=1)).astype(BFNP)
    in_maps = []
    for c in range(num_cores):
        b, s = c // 2, c % 2
        rows = slice(s * NS, (s + 1) * NS)
        orows = slice((1 - s) * NS, (2 - s) * NS)
        ablk = adj[b, rows, :].astype(np.float32)     # [NS, N]
        # own-first column permutation, then transpose -> [N, NS]
        adjT = np.ascontiguousarray(
            np.concatenate([ablk[:, rows], ablk[:, orows]], axis=1).T)
        m = dict(shared)
        m["adjTb"] = adjT.astype(BFNP)
        m["adjT8"] = adjT.astype(FP8NP)
        m["xTs"] = np.ascontiguousarray(
            np.concatenate([x[b, rows].T, x[b, orows].T],
                           axis=1)).astype(BFNP)
        m["xs"] = np.ascontiguousarray(x[b, rows])
        in_maps.append(m)
    return in_maps


_NC_CACHE = {}


def kernel(x, adj, W0, b0, W1, b1, W2, b2, a0, a1, a2, Wt, bt):
    B, N, D = 4, 2048, 128
    H = 256
    NUM_CORES = 8
    NS = N // 2
    pair_groups = [[2 * i, 2 * i + 1] for i in range(NUM_CORES // 2)]

    key = (N, NS, D, H, NUM_CORES)
    if key not in _NC_CACHE:
        _NC_CACHE[key] = build_gat_nc(N, NS, D, H, NUM_CORES, pair_groups)
    nc = _NC_CACHE[key]

    in_maps = make_in_maps(np.asarray(x), np.asarray(adj),
                           [W0, W1, W2], [b0, b1, b2], [a0, a1, a2],
                           np.asarray(Wt), np.asarray(bt), NUM_CORES, NS)
    res = run_bass_kernel_spmd(nc, in_maps, list(range(NUM_CORES))).results
    out = np.empty((B, N, H), np.float32)
    for c in range(NUM_CORES):
        b, s = c // 2, c % 2
        out[b, s * NS:(s + 1) * NS, :] = res[c]["out_s"].astype(np.float32)
    return out


# revision 3
# speedup vs baseline: 1.0428x; 1.0428x over previous
"""GAT (3-layer graph attention + final linear) Trainium2 Bass kernel, v2.

Problem: B=4 graphs, N=2048 atoms, D=128, H=256.
  per layer: h = relu(x @ W.T + b); e_ij = leaky_relu(f1_i + f2_j, 0.01)
  masked by adj; att = softmax_j(e); x = x + att @ h.
  final: relu(x @ Wt.T + bt).

Algorithmic core (validated vs reference at ~4e-3 rel err in numpy):
  exp(leaky_relu(z)) ~= e^z + e^{0.01 z}  (exact at both tails, <=2x off
  only near z=0 where softmax mass is negligible). With z = f1_i + f2_j
  the masked softmax becomes a sum of two rank-1 terms:
    att@h = (num1 + w*num2) / (den1 + w*den2),   w_i = exp(-0.99 f1_i)
    num1 = adj @ (v (.) h), den1 = adj @ v,   v  = exp(f2)
    num2 = adj @ (v'(.) h), den2 = adj @ v',  v' = exp(0.01 f2)
  No NxN exp/mask work remains; the only NxN operand is the adjacency,
  constant across layers, uploaded once (bf16 + fp8 copies, exact 0/1).

Sharding: core c -> (graph c//2, row-half c%2). The SPMD program is
parity-agnostic: the host permutes each core's adjacency columns so its
OWN atoms are j-tiles 0..7 and the partner's are 8..15. Layer 0 uploads
both x halves (no collective). After each layer's x-update the cores
pair-AllGather their updated x half (bf16) and recover the partner half
as slotA + slotB - own (exact); the collective latency is hidden under
the own-half attention work of the next layer.

Engine mapping:
  PE:   h matmul, f1/f2, num1/den (bf16), num2 (fp8 DoubleRow, two
        j-tiles per pass), small row->col transposes for f and den.
  DMA:  XBAR transposes (hsT->hnat, numT->nat, x->xT, final out).
  DVE:  relu+bias, v-scalings of h (stride-0 broadcast APs), batched
        normalize/residual update.
  ACT:  exps only (single LUT set).
"""

import numpy as np
import ml_dtypes

import concourse.bass as bass
import concourse.mybir as mybir
import concourse.tile as tile
from concourse import masks
from concourse.bass_utils import run_bass_kernel_spmd

P = 128
F32 = mybir.dt.float32
BF16 = mybir.dt.bfloat16
FP8 = mybir.dt.float8e4
F16 = mybir.dt.float16
AF = mybir.ActivationFunctionType
OP = mybir.AluOpType
DR = mybir.MatmulPerfMode.DoubleRow
BFNP = ml_dtypes.bfloat16
FP8NP = ml_dtypes.float8_e4m3


def _legalize_waits(nc, dma_limit=1, engine_limit=1):
    """Walrus encodes few sem waits per instr (1 on DMA, 0 on XPOSE DMA,
    ~2 on engine instrs). Move excess waits onto standalone EventSemaphore
    instructions on the same engine."""
    counter = [0]

    def split(ins):
        si = ins.sync_info
        if si is None:
            return None
        tname = type(ins).__name__
        if tname == "InstDmaTransposeAnt":
            limit = 0
        elif tname.startswith("InstDMA") or tname.startswith("InstDma"):
            limit = dma_limit
        else:
            limit = engine_limit
        waits = list(si.on_wait)
        if len(waits) <= limit:
            return None
        keep = waits[-limit:] if limit > 0 else []
        extra = waits[:-limit] if limit > 0 else waits
        evs = []
        for w in extra:
            counter[0] += 1
            evs.append(mybir.InstEventSemaphore(
                name=f"evsplit{counter[0]}", engine=ins.engine,
                sync_info=mybir.SyncInfo(on_wait=[w], on_update=[])))
        ins.sync_info = mybir.SyncInfo(on_wait=keep,
                                       on_update=list(si.on_update))
        return evs

    for f in nc.m.functions:
        for blk in f.blocks:
            new_list = []
            changed = False
            for ins in blk.instructions:
                evs = split(ins)
                if evs:
                    new_list.extend(evs)
                    changed = True
                new_list.append(ins)
            if changed:
                blk.instructions = new_list


def build_gat_nc(N, NS, D, H, num_cores, pair_groups, nlayers=3,
                 legalize=True):
    assert D == P
    nj = N // P          # 16 j tiles (own-first per-core labeling)
    nit = NS // P        # 8 i tiles / own j tiles
    npair = nj // 2      # 8 DoubleRow j-tile pairs
    nH = H // P

    nc = bass.Bass("TRN2", target_bir_lowering=False, debug=False,
                   num_devices=num_cores)

    # ---- I/O ----
    adjTb_in = nc.dram_tensor("adjTb", [N, NS], BF16, kind="ExternalInput")
    adjT8_in = nc.dram_tensor("adjT8", [N, NS], FP8, kind="ExternalInput")
    xTs_in = nc.dram_tensor("xTs", [P, N], BF16, kind="ExternalInput")
    xs_in = nc.dram_tensor("xs", [NS, D], F32, kind="ExternalInput")
    WT_in = [nc.dram_tensor(f"WT{l}", [D, D], BF16, kind="ExternalInput")
             for l in range(nlayers)]
    bv_in = [nc.dram_tensor(f"bv{l}", [D, 1], F32, kind="ExternalInput")
             for l in range(nlayers)]
    av_in = [nc.dram_tensor(f"av{l}", [D, 2], BF16, kind="ExternalInput")
             for l in range(nlayers)]
    WtT_in = nc.dram_tensor("WtT", [D, H], BF16, kind="ExternalInput")
    btp_in = nc.dram_tensor("btp", [P, nH], F32, kind="ExternalInput")
    out_ext = nc.dram_tensor("out_s", [NS, H], BF16, kind="ExternalOutput")

    # DRAM bounce buffers for the pair AllGather of x halves (bf16).
    # Parity-free partner recovery: partner = slotA + slotB - own (exact
    # in f32 arithmetic on bf16 values). Layer 0 needs no exchange: both
    # halves of the initial x are uploaded.
    ag_in = [nc.dram_tensor(f"ag_in{l}", [P, NS], BF16)
             for l in range(1, nlayers)]
    ag_out = [nc.dram_tensor(f"ag_out{l}", [2 * P, NS], BF16)
              for l in range(1, nlayers)]

    with tile.TileContext(nc) as tc:
        import contextlib
        ctx = contextlib.ExitStack()
        with ctx:
            persist = ctx.enter_context(tc.tile_pool(name="persist", bufs=1))
            hsp = ctx.enter_context(tc.tile_pool(name="hsp", bufs=2))
            natp = ctx.enter_context(tc.tile_pool(name="natp", bufs=2))
            xsp = ctx.enter_context(tc.tile_pool(name="xsp", bufs=2))
            xtp = ctx.enter_context(tc.tile_pool(name="xtp", bufs=2))
            smallp = ctx.enter_context(tc.tile_pool(name="smallp", bufs=2))
            nsbp = ctx.enter_context(tc.tile_pool(name="nsbp", bufs=2))
            osbp = ctx.enter_context(tc.tile_pool(name="osbp", bufs=2))
            poolA = ctx.enter_context(
                tc.tile_pool(name="poolA", bufs=2, space="PSUM"))
            pn1 = ctx.enter_context(
                tc.tile_pool(name="pn1", bufs=1, space="PSUM"))
            pn2 = ctx.enter_context(
                tc.tile_pool(name="pn2", bufs=1, space="PSUM"))
            pden = ctx.enter_context(
                tc.tile_pool(name="pden", bufs=1, space="PSUM"))

            ident = persist.tile([P, P], F32)
            masks.make_identity(nc, ident[:])

            # ---- persistent data (small tensors first: layer-0's h
            # matmul must not wait behind the 6MB adjacency upload) ----
            WT = [persist.tile([D, D], BF16, name=f"WT{l}", tag=f"WT{l}")
                  for l in range(nlayers)]
            bv = [persist.tile([D, 1], F32, name=f"bv{l}", tag=f"bv{l}")
                  for l in range(nlayers)]
            av = [persist.tile([D, 2], BF16, name=f"av{l}", tag=f"av{l}")
                  for l in range(nlayers)]
            xTs = [xtp.tile([P, NS], BF16, name=f"xTs0_{hh}",
                            tag=f"xTs{hh}") for hh in range(2)]
            nc.sync.dma_start(xTs[0][:], xTs_in.ap()[:, 0:NS])
            nc.sync.dma_start(xTs[1][:], xTs_in.ap()[:, NS:N])
            for l in range(nlayers):
                nc.sync.dma_start(WT[l][:], WT_in[l].ap())
                nc.sync.dma_start(bv[l][:], bv_in[l].ap())
                nc.sync.dma_start(av[l][:], av_in[l].ap())

            # per-quarter adjacency tiles: dependency tracking is
            # tile-granular, so aggregation on j-tile q must not wait for
            # later quarters' uploads
            adjTb = [persist.tile([P, 4, NS], BF16, name=f"adjTb{q}",
                                  tag=f"adjTb{q}") for q in range(nj // 4)]
            adjT8 = [persist.tile([P, 4, NS], FP8, name=f"adjT8{q}",
                                  tag=f"adjT8{q}") for q in range(nj // 4)]
            adjb_src = adjTb_in.ap().rearrange("(a p) i -> p a i", p=P)
            adj8_src = adjT8_in.ap().rearrange("(a p) i -> p a i", p=P)
            for q in range(nj // 4):
                sl = slice(4 * q, 4 * q + 4)
                nc.sync.dma_start(adjTb[q][:], adjb_src[:, sl, :])
                nc.sync.dma_start(adjT8[q][:], adj8_src[:, sl, :])

            # needed only mid-layer-0 / at the end: after the adj bulk
            xs = xsp.tile([P, nit, P], F32, name="xs0", tag="xs")
            nc.sync.dma_start(
                xs[:], xs_in.ap().rearrange("(a p) d -> p a d", p=P))
            WtT = persist.tile([D, H], BF16)
            nc.sync.dma_start(WtT[:], WtT_in.ap())
            btp = persist.tile([P, nH], F32)
            nc.sync.dma_start(btp[:], btp_in.ap())

            slA = slB = None
            for l in range(nlayers):
                hsT = [hsp.tile([P, NS], BF16, name=f"hsT{l}_{hh}",
                                tag=f"hsT{hh}") for hh in range(2)]
                hnat = [natp.tile([P, nit, P], BF16, name=f"hnat{l}_{hh}",
                                  tag=f"hnat{hh}") for hh in range(2)]
                g1 = [natp.tile([P, nit, P], BF16, name=f"g1_{l}_{hh}",
                                tag=f"g1_{hh}") for hh in range(2)]
                g2 = [natp.tile([P, nit, P], FP8, name=f"g2_{l}_{hh}",
                                tag=f"g2_{hh}") for hh in range(2)]
                frow = [smallp.tile([2, NS], F32, name=f"frow{l}_{hh}",
                                    tag=f"frow{hh}") for hh in range(2)]
                fcol = [smallp.tile([P, nit, 2], F32, name=f"fcol{l}_{hh}",
                                    tag=f"fcol{hh}") for hh in range(2)]
                vv = [smallp.tile([P, nit, 2], BF16, name=f"vv{l}_{hh}",
                                  tag=f"vv{hh}") for hh in range(2)]

                psn1 = pn1.tile([P, NS], F32, name=f"psn1_{l}", tag="n1")
                psn2 = pn2.tile([P, NS], F32, name=f"psn2_{l}", tag="n2")
                psden = pden.tile([2, NS], F32, name=f"psden{l}", tag="den")

                def h_half(hh):
                    """hsT[hh] = relu(WT^T @ xTs[hh] + b), bf16."""
                    for c in range(2):
                        src_ap = xTs[hh][:, c * 512:(c + 1) * 512]
                        ph = poolA.tile([P, 512], F32,
                                        name=f"ph{l}_{hh}_{c}", tag="A")
                        nc.tensor.matmul(ph[:], WT[l][:], src_ap,
                                         start=True, stop=True)
                        nc.vector.tensor_scalar(
                            hsT[hh][:, c * 512:(c + 1) * 512], ph[:],
                            bv[l][:], 0.0, OP.add, OP.max)

                def half_prep(hh):
                    """f1f2 + row->col transposes (PE) + exps + hnat + g
                    scalings for half hh (0 = own rows, 1 = partner)."""
                    for c in range(2):
                        pf = poolA.tile([2, 512], F32,
                                        name=f"pf{l}_{hh}_{c}", tag="A")
                        nc.tensor.matmul(
                            pf[:], av[l][:],
                            hsT[hh][:, c * 512:(c + 1) * 512],
                            start=True, stop=True)
                        nc.scalar.activation(
                            frow[hh][:, c * 512:(c + 1) * 512], pf[:],
                            AF.Copy)
                    pt = poolA.tile([P, 16], F32, name=f"pt{l}_{hh}",
                                    tag="A")
                    for q in range(nit):
                        nc.tensor.transpose(
                            pt[:, 2 * q:2 * q + 2],
                            frow[hh][:, q * P:(q + 1) * P], ident[0:2, 0:2])
                    nc.scalar.activation(
                        fcol[hh][:].rearrange("p a b -> p (a b)"), pt[:],
                        AF.Copy)
                    nc.scalar.activation(vv[hh][:, :, 0],
                                         fcol[hh][:, :, 1], AF.Exp)
                    nc.scalar.activation(vv[hh][:, :, 1],
                                         fcol[hh][:, :, 1], AF.Exp,
                                         scale=0.01)
                    nc.sync.dma_start(hnat[hh][:], hsT[hh][:],
                                      transpose=True)
                    vb = vv[hh][:, :, 0:1].broadcast_to([P, nit, P])
                    nc.vector.tensor_tensor(g1[hh][:], hnat[hh][:], vb,
                                            OP.mult)
                    vpb = vv[hh][:, :, 1:2].broadcast_to([P, nit, P])
                    nc.vector.tensor_tensor(g2[hh][:], hnat[hh][:], vpb,
                                            OP.mult)

                def half_agg(hh, first, last):
                    """num1/den bf16 + num2 fp8-DR streams for half hh."""
                    for q in range(nit):
                        aq, aj = (2 * hh + q // 4), q % 4
                        for c in range(2):
                            sl = slice(c * 512, (c + 1) * 512)
                            nc.tensor.matmul(
                                psn1[:, sl], g1[hh][:, q, :],
                                adjTb[aq][:, aj, sl],
                                start=(first and q == 0),
                                stop=(last and q == nit - 1))
                            nc.tensor.matmul(
                                psden[:, sl], vv[hh][:, q, :],
                                adjTb[aq][:, aj, sl],
                                start=(first and q == 0),
                                stop=(last and q == nit - 1))
                    for k in range(npair // 2):
                        aq, ak = (2 * hh + k // 2), k % 2
                        for c in range(2):
                            sl = slice(c * 512, (c + 1) * 512)
                            nc.tensor.matmul(
                                psn2[:, sl],
                                g2[hh][:, 2 * k:2 * k + 2, :],
                                adjT8[aq][:, 2 * ak:2 * ak + 2, sl],
                                start=(first and k == 0),
                                stop=(last and k == npair // 2 - 1),
                                perf_mode=DR)

                # own half first (overlaps the partner-x exchange that was
                # launched at the end of the previous layer), then partner
                h_half(0)
                half_prep(0)
                half_agg(0, first=True, last=False)
                if l > 0:
                    # partner x: xTs[1] = slA + slB - own  (exact on bf16)
                    nc.vector.tensor_tensor(slA[:], slA[:], slB[:], OP.add)
                    nc.vector.tensor_tensor(xTs[1][:], slA[:], xTs[0][:],
                                            OP.subtract)
                h_half(1)
                half_prep(1)
                half_agg(1, first=False, last=True)

                # num psum -> bf16 sbuf first (longest downstream chain)
                nsb1 = nsbp.tile([P, NS], BF16, name=f"nsb1_{l}",
                                 tag="nsb1")
                nc.scalar.activation(nsb1[:], psn1[:], AF.Copy)
                nsb2 = nsbp.tile([P, NS], BF16, name=f"nsb2_{l}",
                                 tag="nsb2")
                nc.vector.tensor_copy(nsb2[:], psn2[:])
                n1nat = natp.tile([P, nit, P], BF16, name=f"n1nat{l}",
                                  tag="n1nat")
                nc.sync.dma_start(n1nat[:], nsb1[:], transpose=True)
                n2nat = natp.tile([P, nit, P], BF16, name=f"n2nat{l}",
                                  tag="n2nat")
                nc.sync.dma_start(n2nat[:], nsb2[:], transpose=True)

                # -- den: psum [2, NS] -> dencol [P, nit, 2] --
                denrow = smallp.tile([2, NS], F32, name=f"denrow{l}",
                                     tag="denrow")
                nc.scalar.activation(denrow[:], psden[:], AF.Copy)
                pd = poolA.tile([P, 16], F32, name=f"pd{l}", tag="A")
                for q in range(nit):
                    nc.tensor.transpose(pd[:, 2 * q:2 * q + 2],
                                        denrow[:, q * P:(q + 1) * P],
                                        ident[0:2, 0:2])
                dencol = smallp.tile([P, nit, 2], F32, name=f"dencol{l}",
                                     tag="dencol")
                nc.scalar.activation(
                    dencol[:].rearrange("p a b -> p (a b)"), pd[:], AF.Copy)
                wcol = smallp.tile([P, nit], F32, name=f"wcol{l}",
                                   tag="wcol")
                nc.scalar.activation(wcol[:], fcol[0][:, :, 0],
                                     AF.Exp, scale=-0.99)

                # -- r = 1/(den1 + w den2), rw = r*w --
                dtot = smallp.tile([P, nit], F32, name=f"dtot{l}",
                                   tag="dtot")
                nc.vector.tensor_tensor(dtot[:], dencol[:, :, 1], wcol[:],
                                        OP.mult)
                nc.vector.tensor_tensor(dtot[:], dtot[:], dencol[:, :, 0],
                                        OP.add)
                rcol = smallp.tile([P, nit], F32, name=f"rcol{l}",
                                   tag="rcol")
                nc.vector.reciprocal(rcol[:], dtot[:])
                rwcol = smallp.tile([P, nit], F32, name=f"rwcol{l}",
                                    tag="rwcol")
                nc.vector.tensor_tensor(rwcol[:], rcol[:], wcol[:], OP.mult)

                # -- x update (batched, stride-0 free-dim broadcasts) --
                t1 = xsp.tile([P, nit, P], F32, name=f"t1_{l}", tag="t1")
                nc.vector.tensor_tensor(
                    t1[:], n1nat[:],
                    rcol[:].unsqueeze(2).broadcast_to([P, nit, P]), OP.mult)
                t2 = xsp.tile([P, nit, P], F32, name=f"t2_{l}", tag="t2")
                nc.vector.tensor_tensor(
                    t2[:], n2nat[:],
                    rwcol[:].unsqueeze(2).broadcast_to([P, nit, P]),
                    OP.mult)
                nc.vector.tensor_tensor(t1[:], t1[:], t2[:], OP.add)
                xs_new = xsp.tile([P, nit, P], F32, name=f"xs{l + 1}",
                                  tag="xs")
                nc.vector.tensor_tensor(xs_new[:], t1[:], xs[:], OP.add)
                xs = xs_new

                # -- xTs for next layer / final --
                xb = xtp.tile([P, nit, P], BF16, name=f"xb{l}", tag="xb")
                nc.scalar.activation(
                    xb[:].rearrange("p a b -> p (a b)"),
                    xs[:].rearrange("p a b -> p (a b)"), AF.Copy)
                xTs_new = xtp.tile([P, NS], BF16, name=f"xTs{l + 1}",
                                   tag="xTs0")
                nc.sync.dma_start(
                    xTs_new[:].rearrange("p (a b) -> p a b", b=P),
                    xb[:].rearrange("p a b -> p (a b)"), transpose=True)

                if l < nlayers - 1:
                    slA = hsp.tile([P, NS], BF16, name=f"slA{l + 1}",
                                   tag="slA")
                    slB = hsp.tile([P, NS], BF16, name=f"slB{l + 1}",
                                   tag="slB")
                    nc.gpsimd.dma_start(ag_in[l].ap(), xTs_new[:])
                    nc.gpsimd.collective_compute(
                        "AllGather", OP.bypass, replica_groups=pair_groups,
                        ins=[ag_in[l].ap()], outs=[ag_out[l].ap()])
                    nc.gpsimd.dma_start(slA[:], ag_out[l].ap()[0:P, :])
                    nc.gpsimd.dma_start(slB[:], ag_out[l].ap()[P:2 * P, :])
                    xTs = [xTs_new,
                           xtp.tile([P, NS], BF16, name=f"xTs{l + 1}_1",
                                    tag="xTs1")]
                else:
                    xTs = [xTs_new, None]

            # ---- final linear, transposed: outT = relu(WtT^T @ xTs + bt)
            onat = natp.tile([P, nit, nH, P], BF16, name="onat", tag="onat")
            for t in range(nH):
                osbT = osbp.tile([P, NS], BF16, name=f"osbT{t}",
                                 tag=f"osbT{t}")
                for c in range(2):
                    po = poolA.tile([P, 512], F32, name=f"po{t}_{c}",
                                    tag="A")
                    nc.tensor.matmul(po[:], WtT[:, t * P:(t + 1) * P],
                                     xTs[0][:, c * 512:(c + 1) * 512],
                                     start=True, stop=True)
                    nc.vector.tensor_scalar(osbT[:, c * 512:(c + 1) * 512],
                                            po[:], btp[:, t:t + 1], 0.0,
                                            OP.add, OP.max)
                nc.sync.dma_start(onat[:, :, t, :], osbT[:],
                                  transpose=True)
            nc.sync.dma_start(
                out_ext.ap().rearrange("(a p) (t q) -> p a t q", p=P, q=P),
                onat[:])

    if legalize:
        _legalize_waits(nc)
    return nc


def make_in_maps(x, adj, Ws, bs, avs, Wt, bt, num_cores, NS):
    """Per-core input dicts. Core c -> (graph c//2, row-half c%2).
    adjT columns (j) are permuted own-rows-first per core."""
    B, N, D = x.shape
    H = Wt.shape[0]
    x = np.ascontiguousarray(np.asarray(x), np.float32)
    adj = np.asarray(adj)
    shared = {
        "WtT": np.ascontiguousarray(
            np.asarray(Wt, np.float32).T).astype(BFNP),
        "btp": np.ascontiguousarray(
            np.asarray(bt, np.float32).reshape(H // P, P).T),
    }
    for l, (W, b, a) in enumerate(zip(Ws, bs, avs)):
        shared[f"WT{l}"] = np.ascontiguousarray(
            np.asarray(W, np.float32).T).astype(BFNP)
        shared[f"bv{l}"] = np.ascontiguousarray(
            np.asarray(b, np.float32).reshape(D, 1))
        a = np.asarray(a, np.float32)
        shared[f"av{l}"] = np.ascontiguousarray(
            np.stack([a[:D, 0], a[D:, 0]], axis=1)).astype(BFNP)
    in_maps = []
    for c in range(num_cores):
        b, s = c // 2, c % 2
        rows = slice(s * NS, (s + 1) * NS)
        orows = slice((1 - s) * NS, (2 - s) * NS)
        ablk = adj[b, rows, :].astype(np.float32)     # [NS, N]
        # own-first column permutation, then transpose -> [N, NS]
        adjT = np.ascontiguousarray(
            np.concatenate([ablk[:, rows], ablk[:, orows]], axis=1).T)
        m = dict(shared)
        m["adjTb"] = adjT.astype(BFNP)
        m["adjT8"] = adjT.astype(FP8NP)
        m["xTs"] = np.ascontiguousarray(
            np.concatenate([x[b, rows].T, x[b, orows].T],
                           axis=1)).astype(BFNP)
        m["xs"] = np.ascontiguousarray(x[b, rows])
        in_maps.append(m)
    return in_maps


_NC_CACHE = {}


def kernel(x, adj, W0, b0, W1, b1, W2, b2, a0, a1, a2, Wt, bt):
    B, N, D = 4, 2048, 128
    H = 256
    NUM_CORES = 8
    NS = N // 2
    pair_groups = [[2 * i, 2 * i + 1] for i in range(NUM_CORES // 2)]

    key = (N, NS, D, H, NUM_CORES)
    if key not in _NC_CACHE:
        _NC_CACHE[key] = build_gat_nc(N, NS, D, H, NUM_CORES, pair_groups)
    nc = _NC_CACHE[key]

    in_maps = make_in_maps(np.asarray(x), np.asarray(adj),
                           [W0, W1, W2], [b0, b1, b2], [a0, a1, a2],
                           np.asarray(Wt), np.asarray(bt), NUM_CORES, NS)
    res = run_bass_kernel_spmd(nc, in_maps, list(range(NUM_CORES))).results
    out = np.empty((B, N, H), np.float32)
    for c in range(NUM_CORES):
        b, s = c // 2, c % 2
        out[b, s * NS:(s + 1) * NS, :] = res[c]["out_s"].astype(np.float32)
    return out


# revision 4
# speedup vs baseline: 1.1106x; 1.0650x over previous
"""GAT (3-layer graph attention + final linear) Trainium2 Bass kernel, v2.

Problem: B=4 graphs, N=2048 atoms, D=128, H=256.
  per layer: h = relu(x @ W.T + b); e_ij = leaky_relu(f1_i + f2_j, 0.01)
  masked by adj; att = softmax_j(e); x = x + att @ h.
  final: relu(x @ Wt.T + bt).

Algorithmic core (validated vs reference at ~4e-3 rel err in numpy):
  exp(leaky_relu(z)) ~= e^z + e^{0.01 z}  (exact at both tails, <=2x off
  only near z=0 where softmax mass is negligible). With z = f1_i + f2_j
  the masked softmax becomes a sum of two rank-1 terms:
    att@h = (num1 + w*num2) / (den1 + w*den2),   w_i = exp(-0.99 f1_i)
    num1 = adj @ (v (.) h), den1 = adj @ v,   v  = exp(f2)
    num2 = adj @ (v'(.) h), den2 = adj @ v',  v' = exp(0.01 f2)
  No NxN exp/mask work remains; the only NxN operand is the adjacency,
  constant across layers, uploaded once (bf16 + fp8 copies, exact 0/1).

Sharding: core c -> (graph c//2, row-half c%2). The SPMD program is
parity-agnostic: the host permutes each core's adjacency columns so its
OWN atoms are j-tiles 0..7 and the partner's are 8..15. Layer 0 uploads
both x halves (no collective). After each layer's x-update the cores
pair-AllGather their updated x half (bf16) and recover the partner half
as slotA + slotB - own (exact); the collective latency is hidden under
the own-half attention work of the next layer.

Engine mapping:
  PE:   h matmul, f1/f2, num1/den (bf16), num2 (fp8 DoubleRow, two
        j-tiles per pass), small row->col transposes for f and den.
  DMA:  XBAR transposes (hsT->hnat, numT->nat, x->xT, final out).
  DVE:  relu+bias, v-scalings of h (stride-0 broadcast APs), batched
        normalize/residual update.
  ACT:  exps only (single LUT set).
"""

import numpy as np
import ml_dtypes

import concourse.bass as bass
import concourse.mybir as mybir
import concourse.tile as tile
from concourse import masks
from concourse.bass_utils import run_bass_kernel_spmd

P = 128
F32 = mybir.dt.float32
BF16 = mybir.dt.bfloat16
FP8 = mybir.dt.float8e4
F16 = mybir.dt.float16
AF = mybir.ActivationFunctionType
OP = mybir.AluOpType
DR = mybir.MatmulPerfMode.DoubleRow
BFNP = ml_dtypes.bfloat16
FP8NP = ml_dtypes.float8_e4m3


def _legalize_waits(nc, dma_limit=1, engine_limit=1):
    """Walrus encodes few sem waits per instr (1 on DMA, 0 on XPOSE DMA,
    ~2 on engine instrs). Move excess waits onto standalone EventSemaphore
    instructions on the same engine."""
    counter = [0]

    def split(ins):
        si = ins.sync_info
        if si is None:
            return None
        tname = type(ins).__name__
        if tname == "InstDmaTransposeAnt":
            limit = 0
        elif tname.startswith("InstDMA") or tname.startswith("InstDma"):
            limit = dma_limit
        else:
            limit = engine_limit
        waits = list(si.on_wait)
        if len(waits) <= limit:
            return None
        keep = waits[-limit:] if limit > 0 else []
        extra = waits[:-limit] if limit > 0 else waits
        evs = []
        for w in extra:
            counter[0] += 1
            evs.append(mybir.InstEventSemaphore(
                name=f"evsplit{counter[0]}", engine=ins.engine,
                sync_info=mybir.SyncInfo(on_wait=[w], on_update=[])))
        ins.sync_info = mybir.SyncInfo(on_wait=keep,
                                       on_update=list(si.on_update))
        return evs

    for f in nc.m.functions:
        for blk in f.blocks:
            new_list = []
            changed = False
            for ins in blk.instructions:
                evs = split(ins)
                if evs:
                    new_list.extend(evs)
                    changed = True
                new_list.append(ins)
            if changed:
                blk.instructions = new_list


def build_gat_nc(N, NS, D, H, num_cores, pair_groups, nlayers=3,
                 legalize=True):
    assert D == P
    nj = N // P          # 16 j tiles (own-first per-core labeling)
    nit = NS // P        # 8 i tiles / own j tiles
    npair = nj // 2      # 8 DoubleRow j-tile pairs
    nH = H // P

    nc = bass.Bass("TRN2", target_bir_lowering=False, debug=False,
                   num_devices=num_cores)

    # ---- I/O ----
    adjTb_in = nc.dram_tensor("adjTb", [N, NS], BF16, kind="ExternalInput")
    adjT8_in = nc.dram_tensor("adjT8", [N, NS], FP8, kind="ExternalInput")
    xTs_in = nc.dram_tensor("xTs", [P, N], BF16, kind="ExternalInput")
    xs_in = nc.dram_tensor("xs", [NS, D], F32, kind="ExternalInput")
    WT_in = [nc.dram_tensor(f"WT{l}", [D, D], BF16, kind="ExternalInput")
             for l in range(nlayers)]
    bv_in = [nc.dram_tensor(f"bv{l}", [D, 1], F32, kind="ExternalInput")
             for l in range(nlayers)]
    av_in = [nc.dram_tensor(f"av{l}", [D, 2], BF16, kind="ExternalInput")
             for l in range(nlayers)]
    WtT_in = nc.dram_tensor("WtT", [D, H], BF16, kind="ExternalInput")
    btp_in = nc.dram_tensor("btp", [P, nH], F32, kind="ExternalInput")
    out_ext = nc.dram_tensor("out_s", [NS, H], BF16, kind="ExternalOutput")

    # DRAM bounce buffers for the pair AllGather of x halves (bf16).
    # Parity-free partner recovery: partner = slotA + slotB - own (exact
    # in f32 arithmetic on bf16 values). Layer 0 needs no exchange: both
    # halves of the initial x are uploaded.
    ag_in = [nc.dram_tensor(f"ag_in{l}", [P, NS], BF16)
             for l in range(1, nlayers)]
    ag_out = [nc.dram_tensor(f"ag_out{l}", [2 * P, NS], BF16)
              for l in range(1, nlayers)]


    with tile.TileContext(nc) as tc:
        import contextlib
        ctx = contextlib.ExitStack()
        with ctx:
            persist = ctx.enter_context(tc.tile_pool(name="persist", bufs=1))
            hsp = ctx.enter_context(tc.tile_pool(name="hsp", bufs=2))
            natp = ctx.enter_context(tc.tile_pool(name="natp", bufs=2))
            xsp = ctx.enter_context(tc.tile_pool(name="xsp", bufs=2))
            xtp = ctx.enter_context(tc.tile_pool(name="xtp", bufs=2))
            smallp = ctx.enter_context(tc.tile_pool(name="smallp", bufs=2))
            nsbp = ctx.enter_context(tc.tile_pool(name="nsbp", bufs=2))
            osbp = ctx.enter_context(tc.tile_pool(name="osbp", bufs=2))
            poolA = ctx.enter_context(
                tc.tile_pool(name="poolA", bufs=2, space="PSUM"))
            pn1 = ctx.enter_context(
                tc.tile_pool(name="pn1", bufs=1, space="PSUM"))
            pn2 = ctx.enter_context(
                tc.tile_pool(name="pn2", bufs=1, space="PSUM"))
            pden = ctx.enter_context(
                tc.tile_pool(name="pden", bufs=1, space="PSUM"))

            ident = persist.tile([P, P], F32)
            masks.make_identity(nc, ident[:])
            identb = persist.tile([P, P], BF16)
            masks.make_identity(nc, identb[:])


            # ---- persistent data (small tensors first: layer-0's h
            # matmul must not wait behind the 6MB adjacency upload) ----
            WT = [persist.tile([D, D], BF16, name=f"WT{l}", tag=f"WT{l}")
                  for l in range(nlayers)]
            bv = [persist.tile([D, 1], F32, name=f"bv{l}", tag=f"bv{l}")
                  for l in range(nlayers)]
            av = [persist.tile([D, 2], BF16, name=f"av{l}", tag=f"av{l}")
                  for l in range(nlayers)]
            xTs = [xtp.tile([P, NS], BF16, name=f"xTs0_{hh}",
                            tag=f"xTs{hh}") for hh in range(2)]
            nc.sync.dma_start(xTs[0][:], xTs_in.ap()[:, 0:NS])
            nc.sync.dma_start(xTs[1][:], xTs_in.ap()[:, NS:N])
            for l in range(nlayers):
                nc.sync.dma_start(WT[l][:], WT_in[l].ap())
                nc.sync.dma_start(bv[l][:], bv_in[l].ap())
                nc.sync.dma_start(av[l][:], av_in[l].ap())

            # per-quarter adjacency tiles: dependency tracking is
            # tile-granular, so aggregation on j-tile q must not wait for
            # later quarters' uploads
            adjTb = [persist.tile([P, 4, NS], BF16, name=f"adjTb{q}",
                                  tag=f"adjTb{q}") for q in range(nj // 4)]
            adjT8 = [persist.tile([P, 4, NS], FP8, name=f"adjT8{q}",
                                  tag=f"adjT8{q}") for q in range(nj // 4)]
            adjb_src = adjTb_in.ap().rearrange("(a p) i -> p a i", p=P)
            adj8_src = adjT8_in.ap().rearrange("(a p) i -> p a i", p=P)
            for q, eight in ((0, False), (1, False), (0, True), (1, True),
                             (2, False), (3, False), (2, True), (3, True)):
                sl = slice(4 * q, 4 * q + 4)
                if eight:
                    nc.sync.dma_start(adjT8[q][:], adj8_src[:, sl, :])
                else:
                    nc.sync.dma_start(adjTb[q][:], adjb_src[:, sl, :])

            # needed only mid-layer-0 / at the end: after the adj bulk
            xs = xsp.tile([P, nit, P], F32, name="xs0", tag="xs")
            nc.sync.dma_start(
                xs[:], xs_in.ap().rearrange("(a p) d -> p a d", p=P))
            WtT = persist.tile([D, H], BF16)
            nc.sync.dma_start(WtT[:], WtT_in.ap())
            btp = persist.tile([P, nH], F32)
            nc.sync.dma_start(btp[:], btp_in.ap())

            slA = slB = None
            for l in range(nlayers):
                hsT = [hsp.tile([P, NS], BF16, name=f"hsT{l}_{hh}",
                                tag=f"hsT{hh}") for hh in range(2)]
                hnat = [natp.tile([P, nit, P], BF16, name=f"hnat{l}_{hh}",
                                  tag=f"hnat{hh}") for hh in range(2)]
                g1 = [natp.tile([P, nit, P], BF16, name=f"g1_{l}_{hh}",
                                tag=f"g1_{hh}") for hh in range(2)]
                g2 = [natp.tile([P, nit, P], FP8, name=f"g2_{l}_{hh}",
                                tag=f"g2_{hh}") for hh in range(2)]
                frow = [smallp.tile([2, NS], F32, name=f"frow{l}_{hh}",
                                    tag=f"frow{hh}") for hh in range(2)]
                fcol = [smallp.tile([P, nit, 2], F32, name=f"fcol{l}_{hh}",
                                    tag=f"fcol{hh}") for hh in range(2)]
                vv = [smallp.tile([P, nit, 2], BF16, name=f"vv{l}_{hh}",
                                  tag=f"vv{hh}") for hh in range(2)]

                psn1 = pn1.tile([P, NS], F32, name=f"psn1_{l}", tag="n1")
                psn2 = pn2.tile([P, NS], F32, name=f"psn2_{l}", tag="n2")
                psden = pden.tile([2, NS], F32, name=f"psden{l}", tag="den")

                def h_half(hh):
                    """hsT[hh] = relu(WT^T @ xTs[hh] + b), bf16."""
                    for c in range(2):
                        src_ap = xTs[hh][:, c * 512:(c + 1) * 512]
                        ph = poolA.tile([P, 512], F32,
                                        name=f"ph{l}_{hh}_{c}", tag="A")
                        nc.tensor.matmul(ph[:], WT[l][:], src_ap,
                                         start=True, stop=True)
                        nc.vector.tensor_scalar(
                            hsT[hh][:, c * 512:(c + 1) * 512], ph[:],
                            bv[l][:], 0.0, OP.add, OP.max)

                def half_prep(hh):
                    """f1f2 + row->col transposes (PE) + exps + hnat + g
                    scalings for half hh (0 = own rows, 1 = partner)."""
                    for c in range(2):
                        pf = poolA.tile([2, 512], F32,
                                        name=f"pf{l}_{hh}_{c}", tag="A")
                        nc.tensor.matmul(
                            pf[:], av[l][:],
                            hsT[hh][:, c * 512:(c + 1) * 512],
                            start=True, stop=True)
                        nc.scalar.activation(
                            frow[hh][:, c * 512:(c + 1) * 512], pf[:],
                            AF.Copy)
                    pt = poolA.tile([P, 16], F32, name=f"pt{l}_{hh}",
                                    tag="A")
                    for q in range(nit):
                        nc.tensor.transpose(
                            pt[:, 2 * q:2 * q + 2],
                            frow[hh][:, q * P:(q + 1) * P], ident[0:2, 0:2])
                    nc.scalar.activation(
                        fcol[hh][:].rearrange("p a b -> p (a b)"), pt[:],
                        AF.Copy)
                    nc.scalar.activation(vv[hh][:, :, 0],
                                         fcol[hh][:, :, 1], AF.Exp)
                    nc.scalar.activation(vv[hh][:, :, 1],
                                         fcol[hh][:, :, 1], AF.Exp,
                                         scale=0.01)
                    nc.sync.dma_start(hnat[hh][:], hsT[hh][:],
                                      transpose=True)
                    vb = vv[hh][:, :, 0:1].broadcast_to([P, nit, P])
                    nc.vector.tensor_tensor(g1[hh][:], hnat[hh][:], vb,
                                            OP.mult)
                    vpb = vv[hh][:, :, 1:2].broadcast_to([P, nit, P])
                    nc.vector.tensor_tensor(g2[hh][:], hnat[hh][:], vpb,
                                            OP.mult)

                def half_agg(hh, first, last):
                    """den + num2(fp8-DR) + num1 streams for half hh.
                    Order den->num2->num1 so den/num2 post-processing
                    overlaps the num1 stream at the layer tail."""
                    for q in range(nit):
                        aq, aj = (2 * hh + q // 4), q % 4
                        for c in range(2):
                            sl = slice(c * 512, (c + 1) * 512)
                            nc.tensor.matmul(
                                psden[:, sl], vv[hh][:, q, :],
                                adjTb[aq][:, aj, sl],
                                start=(first and q == 0),
                                stop=(last and q == nit - 1))
                    for k in range(npair // 2):
                        aq, ak = (2 * hh + k // 2), k % 2
                        for c in range(2):
                            sl = slice(c * 512, (c + 1) * 512)
                            nc.tensor.matmul(
                                psn2[:, sl],
                                g2[hh][:, 2 * k:2 * k + 2, :],
                                adjT8[aq][:, 2 * ak:2 * ak + 2, sl],
                                start=(first and k == 0),
                                stop=(last and k == npair // 2 - 1),
                                perf_mode=DR)
                    for q in range(nit):
                        aq, aj = (2 * hh + q // 4), q % 4
                        for c in range(2):
                            sl = slice(c * 512, (c + 1) * 512)
                            nc.tensor.matmul(
                                psn1[:, sl], g1[hh][:, q, :],
                                adjTb[aq][:, aj, sl],
                                start=(first and q == 0),
                                stop=(last and q == nit - 1))

                # own half first (overlaps the partner-x exchange that was
                # launched at the end of the previous layer), then partner
                h_half(0)
                half_prep(0)
                wcol = smallp.tile([P, nit], F32, name=f"wcol{l}",
                                   tag="wcol")
                nc.scalar.activation(wcol[:], fcol[0][:, :, 0],
                                     AF.Exp, scale=-0.99)
                half_agg(0, first=True, last=False)
                if l > 0:
                    # partner x: xTs[1] = slA + slB - own  (exact on bf16)
                    nc.vector.tensor_tensor(slA[:], slA[:], slB[:], OP.add)
                    nc.vector.tensor_tensor(xTs[1][:], slA[:], xTs[0][:],
                                            OP.subtract)
                h_half(1)
                half_prep(1)
                half_agg(1, first=False, last=True)

                # num psum -> bf16 sbuf first (longest downstream chain)
                nsb1 = nsbp.tile([P, NS], BF16, name=f"nsb1_{l}",
                                 tag="nsb1")
                nc.scalar.activation(nsb1[:], psn1[:], AF.Copy)
                nsb2 = nsbp.tile([P, NS], BF16, name=f"nsb2_{l}",
                                 tag="nsb2")
                nc.vector.tensor_copy(nsb2[:], psn2[:])
                n1nat = pn1.tile([P, nit, P], BF16, name=f"n1nat{l}",
                                 tag="n1")
                n2nat = pn2.tile([P, nit, P], BF16, name=f"n2nat{l}",
                                 tag="n2")
                for q in range(nit):
                    nc.tensor.transpose(n1nat[:, q, :],
                                        nsb1[:, q * P:(q + 1) * P],
                                        identb[:])
                for q in range(nit):
                    nc.tensor.transpose(n2nat[:, q, :],
                                        nsb2[:, q * P:(q + 1) * P],
                                        identb[:])

                # -- den: psum [2, NS] -> dencol [P, nit, 2] --
                denrow = smallp.tile([2, NS], F32, name=f"denrow{l}",
                                     tag="denrow")
                nc.scalar.activation(denrow[:], psden[:], AF.Copy)
                pd = poolA.tile([P, 16], F32, name=f"pd{l}", tag="A")
                for q in range(nit):
                    nc.tensor.transpose(pd[:, 2 * q:2 * q + 2],
                                        denrow[:, q * P:(q + 1) * P],
                                        ident[0:2, 0:2])
                dencol = smallp.tile([P, nit, 2], F32, name=f"dencol{l}",
                                     tag="dencol")
                nc.scalar.activation(
                    dencol[:].rearrange("p a b -> p (a b)"), pd[:], AF.Copy)
                # -- r = 1/(den1 + w den2), rw = r*w --
                dtot = smallp.tile([P, nit], F32, name=f"dtot{l}",
                                   tag="dtot")
                nc.vector.tensor_tensor(dtot[:], dencol[:, :, 1], wcol[:],
                                        OP.mult)
                nc.vector.tensor_tensor(dtot[:], dtot[:], dencol[:, :, 0],
                                        OP.add)
                rcol = smallp.tile([P, nit], F32, name=f"rcol{l}",
                                   tag="rcol")
                nc.vector.reciprocal(rcol[:], dtot[:])
                rwcol = smallp.tile([P, nit], F32, name=f"rwcol{l}",
                                    tag="rwcol")
                nc.vector.tensor_tensor(rwcol[:], rcol[:], wcol[:], OP.mult)

                # -- x update (batched, stride-0 free-dim broadcasts) --
                t1 = xsp.tile([P, nit, P], F32, name=f"t1_{l}", tag="t1")
                nc.vector.tensor_tensor(
                    t1[:], n1nat[:],
                    rcol[:].unsqueeze(2).broadcast_to([P, nit, P]), OP.mult)
                t2 = xsp.tile([P, nit, P], F32, name=f"t2_{l}", tag="t2")
                nc.vector.tensor_tensor(
                    t2[:], n2nat[:],
                    rwcol[:].unsqueeze(2).broadcast_to([P, nit, P]),
                    OP.mult)
                nc.vector.tensor_tensor(t1[:], t1[:], t2[:], OP.add)
                xs_new = xsp.tile([P, nit, P], F32, name=f"xs{l + 1}",
                                  tag="xs")
                nc.vector.tensor_tensor(xs_new[:], t1[:], xs[:], OP.add)
                xs = xs_new

                # -- xTs for next layer / final --
                xb = xtp.tile([P, nit, P], BF16, name=f"xb{l}", tag="xb")
                nc.scalar.activation(
                    xb[:].rearrange("p a b -> p (a b)"),
                    xs[:].rearrange("p a b -> p (a b)"), AF.Copy)
                xTs_new = xtp.tile([P, NS], BF16, name=f"xTs{l + 1}",
                                   tag="xTs0")
                nc.sync.dma_start(
                    xTs_new[:].rearrange("p (a b) -> p a b", b=P),
                    xb[:].rearrange("p a b -> p (a b)"), transpose=True)

                if l < nlayers - 1:
                    slA = hsp.tile([P, NS], BF16, name=f"slA{l + 1}",
                                   tag="slA")
                    slB = hsp.tile([P, NS], BF16, name=f"slB{l + 1}",
                                   tag="slB")
                    nc.gpsimd.dma_start(ag_in[l].ap(), xTs_new[:])
                    nc.gpsimd.collective_compute(
                        "AllGather", OP.bypass, replica_groups=pair_groups,
                        ins=[ag_in[l].ap()], outs=[ag_out[l].ap()])
                    nc.gpsimd.dma_start(slA[:], ag_out[l].ap()[0:P, :])
                    nc.gpsimd.dma_start(slB[:], ag_out[l].ap()[P:2 * P, :])
                    xTs = [xTs_new,
                           xtp.tile([P, NS], BF16, name=f"xTs{l + 1}_1",
                                    tag="xTs1")]
                else:
                    xTs = [xTs_new, None]

            # ---- final linear, transposed: outT = relu(WtT^T @ xTs + bt)
            onat = natp.tile([P, nit, nH, P], BF16, name="onat", tag="onat")
            for t in range(nH):
                osbT = osbp.tile([P, NS], BF16, name=f"osbT{t}",
                                 tag=f"osbT{t}")
                for c in range(2):
                    po = poolA.tile([P, 512], F32, name=f"po{t}_{c}",
                                    tag="A")
                    nc.tensor.matmul(po[:], WtT[:, t * P:(t + 1) * P],
                                     xTs[0][:, c * 512:(c + 1) * 512],
                                     start=True, stop=True)
                    nc.vector.tensor_scalar(osbT[:, c * 512:(c + 1) * 512],
                                            po[:], btp[:, t:t + 1], 0.0,
                                            OP.add, OP.max)
                nc.sync.dma_start(onat[:, :, t, :], osbT[:],
                                  transpose=True)
            nc.sync.dma_start(
                out_ext.ap().rearrange("(a p) (t q) -> p a t q", p=P, q=P),
                onat[:])

    if legalize:
        _legalize_waits(nc)
    return nc


def make_in_maps(x, adj, Ws, bs, avs, Wt, bt, num_cores, NS):
    """Per-core input dicts. Core c -> (graph c//2, row-half c%2).
    adjT columns (j) are permuted own-rows-first per core."""
    B, N, D = x.shape
    H = Wt.shape[0]
    x = np.ascontiguousarray(np.asarray(x), np.float32)
    adj = np.asarray(adj)
    shared = {
        "WtT": np.ascontiguousarray(
            np.asarray(Wt, np.float32).T).astype(BFNP),
        "btp": np.ascontiguousarray(
            np.asarray(bt, np.float32).reshape(H // P, P).T),
    }
    for l, (W, b, a) in enumerate(zip(Ws, bs, avs)):
        shared[f"WT{l}"] = np.ascontiguousarray(
            np.asarray(W, np.float32).T).astype(BFNP)
        shared[f"bv{l}"] = np.ascontiguousarray(
            np.asarray(b, np.float32).reshape(D, 1))
        a = np.asarray(a, np.float32)
        shared[f"av{l}"] = np.ascontiguousarray(
            np.stack([a[:D, 0], a[D:, 0]], axis=1)).astype(BFNP)
    in_maps = []
    for c in range(num_cores):
        b, s = c // 2, c % 2
        rows = slice(s * NS, (s + 1) * NS)
        orows = slice((1 - s) * NS, (2 - s) * NS)
        ablk = adj[b, rows, :].astype(np.float32)     # [NS, N]
        # own-first column permutation, then transpose -> [N, NS]
        adjT = np.ascontiguousarray(
            np.concatenate([ablk[:, rows], ablk[:, orows]], axis=1).T)
        m = dict(shared)
        m["adjTb"] = adjT.astype(BFNP)
        m["adjT8"] = adjT.astype(FP8NP)
        m["xTs"] = np.ascontiguousarray(
            np.concatenate([x[b, rows].T, x[b, orows].T],
                           axis=1)).astype(BFNP)
        m["xs"] = np.ascontiguousarray(x[b, rows])
        in_maps.append(m)
    return in_maps


_NC_CACHE = {}


def kernel(x, adj, W0, b0, W1, b1, W2, b2, a0, a1, a2, Wt, bt):
    B, N, D = 4, 2048, 128
    H = 256
    NUM_CORES = 8
    NS = N // 2
    pair_groups = [[2 * i, 2 * i + 1] for i in range(NUM_CORES // 2)]

    key = (N, NS, D, H, NUM_CORES)
    if key not in _NC_CACHE:
        _NC_CACHE[key] = build_gat_nc(N, NS, D, H, NUM_CORES, pair_groups)
    nc = _NC_CACHE[key]

    in_maps = make_in_maps(np.asarray(x), np.asarray(adj),
                           [W0, W1, W2], [b0, b1, b2], [a0, a1, a2],
                           np.asarray(Wt), np.asarray(bt), NUM_CORES, NS)
    res = run_bass_kernel_spmd(nc, in_maps, list(range(NUM_CORES))).results
    out = np.empty((B, N, H), np.float32)
    for c in range(NUM_CORES):
        b, s = c // 2, c % 2
        out[b, s * NS:(s + 1) * NS, :] = res[c]["out_s"].astype(np.float32)
    return out
